# revision 1
# baseline (speedup 1.0000x reference)
"""Trainium2 Bass kernel for nn_AutoSparseLinear.

Problem: out[b,h,o] = sum_d gathered[b,h,d] * W[h,o,d] + bias[h,o]
  where gathered[b,h,k*64+w] = x[b, mask[h,k], w]
  x: [512,128,64] f32, mask: [256,4] i64, W: [256,64,256] f32, b: [256,64] f32
  out: [512,256,64] f32

Strategy (expert-style sharding per the hint): split the H_out group dim
8 ways; each core computes 32 groups over the full batch B=512.

The host (this function) does the mask-dependent gather + layout prep in
numpy, producing per-core packed operands so the device program is
identical on all 8 cores (single SPMD NEFF, no mask-dependence in the
program):
  gx  [128, 32*2*512] fp16 — per (group, d-chunk) gathered-and-transposed
       x blocks: slot(h',c)[p, b] = x[b, mask[h, 2c + p//64], p%64]
  wt  [128, 32*2*64]  fp16 — per-chunk transposed weights:
       slot(h',c)[p, o] = W[h, o, c*128+p]
  bb  [128, 16] f32 — bias pairs: col j = concat(b[2j], b[2j+1])

Device per group-pair j (groups 2j, 2j+1 side by side in PE columns):
  psum[0:64, :]   = wt(2j,0).T   @ gx(2j,0)   + wt(2j,1).T   @ gx(2j,1)
  psum[64:128, :] = wt(2j+1,0).T @ gx(2j+1,0) + wt(2j+1,1).T @ gx(2j+1,1)
  out_sb = psum + bias_col (DVE), DMA to DRAM as [128, 512] contiguous.

Compute is fp16 (inputs rounded to fp16, fp32 PSUM accumulation);
outputs stored fp16 and upcast on host.
"""

import numpy as np

import concourse.mybir as mybir
from concourse import bacc
from concourse.tile import TileContext
from concourse.bass_utils import run_bass_kernel_spmd

# Problem shapes (hardcoded per contract)
B = 512
H_IN = 128
W_IN = 64
H_OUT = 256
W_OUT = 64
K = 4
D = K * W_IN  # 256
N_CORES = 8
HG = H_OUT // N_CORES  # 32 groups per core
N_PAIRS = HG // 2  # 16
N_SLICES = 4  # upload pipelining granularity
GROUPS_PER_SLICE = HG // N_SLICES  # 8
SLOTS_PER_SLICE = GROUPS_PER_SLICE * 2  # 16 (group, chunk) slots

F16 = mybir.dt.float16
F32 = mybir.dt.float32


def build_nc(loop: int = 1, mode: str = "full", dma: str = "gpsimd", timing: bool = False):
    """Build the (uniform-across-cores) Bass program.

    loop > 1 wraps the body in a hardware For_i loop — used only for
    steady-state timing.  mode: "full" | "upload" (DMAs only) |
    "compute" (uploads hoisted out of the loop).
    """
    nc = bacc.Bacc(None, target_bir_lowering=False)
    dmae = getattr(nc, dma)
    gx_d = nc.dram_tensor("gx", [128, HG * 2 * B], F16, kind="ExternalInput")
    wt_d = nc.dram_tensor("wt", [128, HG * 2 * W_OUT], F16, kind="ExternalInput")
    bb_d = nc.dram_tensor("bb", [128, N_PAIRS], F32, kind="ExternalInput")
    if timing:
        # Keep HBM out-traffic but avoid shipping 2MB/core back over the
        # axon tunnel per bench call: write to Internal DRAM, expose a
        # tiny sink as the only ExternalOutput.
        out_d = nc.dram_tensor("out", [HG * W_OUT, B], F16)
        sink_d = nc.dram_tensor("sink", [128, 1], F16, kind="ExternalOutput")
    else:
        out_d = nc.dram_tensor("out", [HG * W_OUT, B], F16, kind="ExternalOutput")
        sink_d = None

    gx_cols = SLOTS_PER_SLICE * B  # per-slice gx columns
    wt_cols = SLOTS_PER_SLICE * W_OUT  # per-slice wt columns

    with TileContext(nc) as tc:
        with (
            tc.tile_pool(name="res", bufs=1) as res,
            tc.tile_pool(name="psum", bufs=8, space="PSUM") as psump,
            tc.tile_pool(name="outs", bufs=6) as outp,
        ):

            last_gx = [None]

            def uploads():
                bt = res.tile([128, N_PAIRS], F32, tag="bias")
                dmae.dma_start(out=bt[:], in_=bb_d[:, :])
                gxs = []
                wts = []
                for s in range(N_SLICES):
                    wtile = res.tile([128, wt_cols], F16, tag=f"wt{s}")
                    dmae.dma_start(
                        out=wtile[:], in_=wt_d[:, s * wt_cols : (s + 1) * wt_cols]
                    )
                    gtile = res.tile([128, gx_cols], F16, tag=f"gx{s}")
                    dmae.dma_start(
                        out=gtile[:], in_=gx_d[:, s * gx_cols : (s + 1) * gx_cols]
                    )
                    wts.append(wtile)
                    gxs.append(gtile)
                last_gx[0] = gxs[-1]
                return bt, gxs, wts

            last_ob = [None]

            def compute(bt, gxs, wts):
                for j in range(N_PAIRS):
                    s = (2 * j) // GROUPS_PER_SLICE
                    ps = psump.tile([128, B], F32, tag="ps")
                    for c in range(2):
                        for hh in range(2):  # group 2j+hh -> psum cols 64*hh
                            lg = ((2 * j + hh) - s * GROUPS_PER_SLICE) * 2 + c
                            lhsT = wts[s][:, lg * W_OUT : (lg + 1) * W_OUT]
                            rhs = gxs[s][:, lg * B : (lg + 1) * B]
                            nc.tensor.matmul(
                                ps[64 * hh : 64 * hh + 64, :],
                                lhsT,
                                rhs,
                                start=(c == 0),
                                stop=(c == 1),
                            )
                    ob = outp.tile([128, B], F16, tag="ob")
                    nc.vector.tensor_scalar_add(ob[:], ps[:, :], bt[:, j : j + 1])
                    dmae.dma_start(
                        out=out_d[128 * j : 128 * (j + 1), :], in_=ob[:]
                    )
                    last_ob[0] = ob

            def body(_iv=None):
                args = uploads()
                if mode != "upload":
                    compute(*args)

            if mode == "compute":
                args = uploads()
                if loop > 1:
                    with tc.For_i(0, loop, 1):
                        compute(*args)
                else:
                    compute(*args)
            elif loop > 1:
                with tc.For_i(0, loop, 1):
                    body()
            else:
                body()

            if sink_d is not None:
                # value is irrelevant; NEFF completion waits for all queues
                st = res.tile([128, 1], F16, tag="sinksrc")
                nc.vector.memset(st[:], 0.0)
                dmae.dma_start(out=sink_d[:, :], in_=st[:])

    nc.finalize()
    return nc


def shard_inputs(x, mask, W, b):
    """Host-side gather + layout prep. Returns per-core input dicts."""
    x = np.asarray(x, dtype=np.float32)
    mask = np.asarray(mask)
    W = np.asarray(W, dtype=np.float32)
    b = np.asarray(b, dtype=np.float32)

    xT = np.ascontiguousarray(x.transpose(1, 2, 0))  # [i, w, b]
    in_maps = []
    for q in range(N_CORES):
        h0 = q * HG
        mq = mask[h0 : h0 + HG]  # [HG, 4]
        g = xT[mq]  # [HG, 4, 64, B]
        g = g.reshape(HG, 2, 128, B).transpose(2, 0, 1, 3)  # [128, HG, 2, B]
        gx = np.ascontiguousarray(g.reshape(128, HG * 2 * B)).astype(
            np.float16
        )

        Wq = W[h0 : h0 + HG]  # [HG, 64, 256]
        wt = (
            Wq.transpose(0, 2, 1)  # [HG, d, o]
            .reshape(HG, 2, 128, W_OUT)
            .transpose(2, 0, 1, 3)  # [128, HG, 2, o]
            .reshape(128, HG * 2 * W_OUT)
        )
        wt = np.ascontiguousarray(wt).astype(np.float16)

        bb = np.empty((128, N_PAIRS), np.float32)
        for j in range(N_PAIRS):
            bb[:64, j] = b[h0 + 2 * j]
            bb[64:, j] = b[h0 + 2 * j + 1]

        in_maps.append({"gx": gx, "wt": wt, "bb": bb})
    return in_maps


def assemble_output(results):
    """results: list of per-core dicts with 'out' [HG*W_OUT, B] f32."""
    out = np.empty((B, H_OUT, W_OUT), np.float32)
    for q, r in enumerate(results):
        o = np.asarray(r["out"], dtype=np.float32).reshape(HG, W_OUT, B)  # [h', o, b]
        out[:, q * HG : (q + 1) * HG, :] = o.transpose(2, 0, 1)
    return out


_NC_CACHE = {}


def kernel(x, mask, W, b):
    in_maps = shard_inputs(x, mask, W, b)
    if "nc" not in _NC_CACHE:
        _NC_CACHE["nc"] = build_nc()
    nc = _NC_CACHE["nc"]
    res = run_bass_kernel_spmd(nc, in_maps, core_ids=list(range(N_CORES)))
    return assemble_output(res.results)

